# revision 1
# baseline (speedup 1.0000x reference)
"""GCLSTM (ChebConv-gated LSTM) Trainium2 kernel, 8-core SPMD.

Algorithm notes
---------------
reference computes, per timestep t (T=24) over N=5120 graph nodes:
    gate_g = X_t @ Ws[g] + cheb(H, thetas[g]) + biases      (4 gates)
    cheb(H, th) = H@th0 + (L@H)@th1 + (2L(LH) - H)@th2      (K=3 Chebyshev)
with L the scaled-normalized graph Laplacian (5120x5120, sparse, here
densified).  The Chebyshev basis (U = L@H, V = L^2@H) is shared by all 4
gates, so per step we need exactly ONE dense "mega-prop" [U|V] = [L;L^2]@H
plus the gate matmuls.  Folding:
    gate_g = X_t@Ws[g] + H@(th0-th2) + U@th1 + V@(2*th2) + b
so all gate work is a single [X;H;U;V] (1024) x Theta (1024x1024) matmul.

Sharding: nodes are split across 8 cores (640 each; edges connect
arbitrary nodes, so each core holds the full [L;L^2] column block for its
output rows, resident in SBUF as fp16).  The mega-prop contracts over ALL
5120 nodes, so the full H (node-major, fp16) is re-assembled every step
with two feature-half AllGathers; everything else stays core-local.
To start the mega-prop before the AllGather lands, each core's [L;L^2]
block is stored as 45 contraction tiles: 5 "own-node" tiles (fed from the
locally produced H slice) followed by the 40 global tiles with the own
rows zeroed, so own-node contributions are never double counted.

Precision: L, L^2, H-for-prop, Theta, X are fp16 (PE matmul accumulates
fp32 in PSUM); LSTM cell state C and gate pre-activations stay fp32.
"""
import sys

for _p in ("/opt/trn_rl_repo",):
    if _p not in sys.path:
        sys.path.insert(0, _p)

import numpy as np
import concourse.bass as bass
import concourse.mybir as mybir
import concourse.tile as tile
from concourse import bacc
from concourse.bass_utils import run_bass_kernel_spmd

fp32 = mybir.dt.float32
fp16 = mybir.dt.float16

NCORES = 8
B, T, NTOW, F = 512, 24, 10, 256
N = B * NTOW                  # 5120 nodes
NLOC = N // NCORES            # 640 nodes per core
KT = N // 128                 # 40 contraction tiles over nodes
KLOC = NLOC // 128            # 5 own-node tiles
KT2 = KT + KLOC               # 45 = own tiles first, then zeroed-own global
FT = F // 128                 # 2 feature tiles
GM = (4 * F) // 128           # 8 gate-feature m-tiles
NOUT2 = 2 * NLOC              # 1280 = [U|V] output columns per core
LAMBDA_MAX = 2.0

NCH = [(0, 512), (512, 640)]             # node chunks for gate matmuls
PCH = [(0, 512), (512, 1024), (1024, 1280)]  # [U|V] column chunks

SIG = mybir.ActivationFunctionType.Sigmoid
TANH = mybir.ActivationFunctionType.Tanh

_CACHE = {}


def _build_nc(repeat=1, no_comm=False, own_first=False, split_ag=True, dma_tr=True):
    nc = bacc.Bacc(None, target_bir_lowering=False, num_devices=NCORES)
    nkt = KT2 if own_first else KT
    d_ll2 = nc.dram_tensor("ll2", [nkt, 128, NOUT2], fp16, kind="ExternalInput")
    d_th = nc.dram_tensor("th", [GM, 128, 4 * F], fp16, kind="ExternalInput")
    d_x = nc.dram_tensor("xall", [T, FT, 128, NLOC], fp16, kind="ExternalInput")
    d_bias = nc.dram_tensor("biasv", [GM, 128], fp32, kind="ExternalInput")
    d_h = nc.dram_tensor("hout", [FT, 128, NLOC], fp32, kind="ExternalOutput")
    d_c = nc.dram_tensor("cout", [FT, 128, NLOC], fp32, kind="ExternalOutput")

    with tile.TileContext(nc) as tc:
        with (
            tc.tile_pool(name="const", bufs=1) as constp,
            tc.tile_pool(name="xp", bufs=1) as xp,
            tc.tile_pool(name="gp", bufs=1) as gp,
            tc.tile_pool(name="uvp", bufs=1) as uvp,
            tc.tile_pool(name="hp", bufs=2) as hp,
            tc.tile_pool(name="hnmp", bufs=2) as hnmp,
            tc.tile_pool(name="tmpp", bufs=1) as tmpp,
            tc.tile_pool(name="psg", bufs=4 if dma_tr else 3, space="PSUM") as psg,
            tc.tile_pool(name="psp", bufs=4 if dma_tr else 3, space="PSUM") as psp,
            tc.tile_pool(name="dramio", bufs=2, space="DRAM") as dramp,
        ):
            # ---- resident tensors ----
            sb_ll2 = constp.tile([128, nkt, NOUT2], fp16, tag="ll2")
            sb_th = constp.tile([128, GM, 4 * F], fp16, tag="th")
            sb_bias = constp.tile([128, GM], fp32, tag="bias")
            sb_hfull = constp.tile([128, KT, F], fp16, tag="hfull")
            if not dma_tr:
                from concourse.masks import make_identity
                ident = constp.tile([128, 128], fp16, tag="ident")
                make_identity(nc, ident)
            nc.sync.dma_start(sb_bias, d_bias.rearrange("m p -> p m"))
            # theta in column chunks so step-0 gates can start early
            thv = d_th.rearrange("k p j -> p k j")
            for mc in range(GM):
                cs = slice(mc * 128, (mc + 1) * 128)
                nc.sync.dma_start(sb_th[:, :, cs], thv[:, :, cs])
            x_first = xp.tile([128, FT, NLOC], fp16, tag="x", name="x_first")
            nc.sync.dma_start(x_first, d_x[0].rearrange("f p n -> p f n"))
            for kg in range(nkt // 5):
                ks = slice(kg * 5, (kg + 1) * 5)
                nc.sync.dma_start(
                    sb_ll2[:, ks, :], d_ll2[ks].rearrange("k p j -> p k j"))

            h_fm = None    # current H_i, feature-major [128, FT, NLOC] fp16
            c_fm = None    # current C_i, feature-major fp32
            hnm_prev = None  # own H slice, node-major [128, KLOC, F] fp16

            first_iter = True
            for t in [tt for _r in range(repeat) for tt in range(T)]:
                if first_iter:
                    x_t = x_first
                    first_iter = False
                else:
                    x_t = xp.tile([128, FT, NLOC], fp16, tag="x", name=f"x{t}")
                    nc.sync.dma_start(x_t, d_x[t].rearrange("f p n -> p f n"))
                gacc = gp.tile([128, GM, NLOC], fp32, tag="g", name=f"g{t}")

                def rhs_of(kk, c0, c1, _x=x_t, _h=h_fm):
                    if kk < 2:
                        return _x[:, kk, c0:c1]
                    return _h[:, kk - 2, c0:c1]

                # ---- gate matmul, X(+H) part ----
                kks = (0, 1) if t == 0 else (0, 1, 2, 3)
                for m in range(GM):
                    pss = [
                        psg.tile([128, c1 - c0], fp32, tag="gps",
                                 name=f"gxh{t}_{m}_{ci}")
                        for ci, (c0, c1) in enumerate(NCH)
                    ]
                    for i, kk in enumerate(kks):
                        for ci, (c0, c1) in enumerate(NCH):
                            nc.tensor.matmul(
                                pss[ci],
                                sb_th[:, kk, m * 128:(m + 1) * 128],
                                rhs_of(kk, c0, c1),
                                start=(i == 0), stop=(i == len(kks) - 1))
                    for ci, (c0, c1) in enumerate(NCH):
                        nc.vector.tensor_copy(gacc[:, m, c0:c1], pss[ci])

                if t > 0:
                    # ---- mega-prop: 5 own-node tiles first (no AG needed),
                    # then 40 global tiles (own rows zeroed in ll2) ----
                    u_fm = uvp.tile([128, FT, NLOC], fp16, tag="u", name=f"u{t}")
                    v_fm = uvp.tile([128, FT, NLOC], fp16, tag="v", name=f"v{t}")
                    for m in range(FT):
                        ms = slice(m * 128, (m + 1) * 128)
                        pps = [
                            psp.tile([128, p1 - p0], fp32, tag="pps",
                                     name=f"pps{t}_{m}_{ci}")
                            for ci, (p0, p1) in enumerate(PCH)
                        ]
                        for k in range(nkt):
                            if own_first:
                                lhsT = (hnm_prev[:, k, ms] if k < KLOC
                                        else sb_hfull[:, k - KLOC, ms])
                            else:
                                lhsT = sb_hfull[:, k, ms]
                            for ci, (p0, p1) in enumerate(PCH):
                                nc.tensor.matmul(
                                    pps[ci], lhsT, sb_ll2[:, k, p0:p1],
                                    start=(k == 0), stop=(k == nkt - 1))
                        nc.vector.tensor_copy(u_fm[:, m, 0:512], pps[0])
                        nc.vector.tensor_copy(u_fm[:, m, 512:640], pps[1][:, 0:128])
                        nc.vector.tensor_copy(v_fm[:, m, 0:384], pps[1][:, 128:512])
                        nc.vector.tensor_copy(v_fm[:, m, 384:640], pps[2])

                    # ---- gate matmul, U/V part (accumulate into gacc) ----
                    # even m-tiles first: they feed the ft=0 half of the LSTM,
                    # unblocking the first AllGather half earlier
                    for m in (0, 2, 4, 6, 1, 3, 5, 7):
                        pss = [
                            psg.tile([128, c1 - c0], fp32, tag="gps",
                                     name=f"guv{t}_{m}_{ci}")
                            for ci, (c0, c1) in enumerate(NCH)
                        ]
                        for i, kk in enumerate((4, 5, 6, 7)):
                            src = u_fm if kk < 6 else v_fm
                            for ci, (c0, c1) in enumerate(NCH):
                                nc.tensor.matmul(
                                    pss[ci],
                                    sb_th[:, kk, m * 128:(m + 1) * 128],
                                    src[:, kk % 2, c0:c1],
                                    start=(i == 0), stop=(i == 3))
                        for ci, (c0, c1) in enumerate(NCH):
                            nc.vector.tensor_add(
                                gacc[:, m, c0:c1], gacc[:, m, c0:c1], pss[ci])

                # ---- LSTM cell (feature-major, elementwise), then transpose
                # the fresh H slice and kick the feature-half AllGathers ----
                last = (t == T - 1)
                h_new = hp.tile([128, FT, NLOC], fp32 if last else fp16,
                                tag="h32" if last else "h", name=f"h{t + 1}",
                                bufs=1 if last else None)
                c_new = hp.tile([128, FT, NLOC], fp32, tag="c", name=f"c{t + 1}")
                if not last:
                    hnm = hnmp.tile([128, KLOC, F], fp16, tag="hnm",
                                    name=f"hnm{t}")
                    agins, agouts = [], []
                    nag = FT if split_ag else 1
                    agw = 128 if split_ag else F
                    for ft in range(nag):
                        agins.append(dramp.tile(
                            [NLOC, agw], fp16, tag=f"agin{ft}",
                            name=f"agin{t}_{ft}"))
                        agouts.append(dramp.tile(
                            [N, agw], fp16, tag=f"agout{ft}",
                            addr_space="Shared", name=f"agout{t}_{ft}"))

                def emit_ag(ft):
                    fs = slice(ft * agw, (ft + 1) * agw)
                    nc.sync.dma_start(
                        agins[ft].rearrange("(k p) f -> p k f", p=128),
                        hnm[:, :, fs])
                    if not no_comm:
                        nc.gpsimd.collective_compute(
                            "AllGather",
                            mybir.AluOpType.bypass,
                            replica_groups=[list(range(NCORES))],
                            ins=[agins[ft].opt()],
                            outs=[agouts[ft].opt()],
                        )
                    agv = agouts[ft].rearrange("(k p) f -> p k f", p=128)
                    for kg in range(5):
                        ks = slice(kg * 8, (kg + 1) * 8)
                        nc.sync.dma_start(sb_hfull[:, ks, fs], agv[:, ks, :])
                for ft in range(FT):
                    ti = tmpp.tile([128, NLOC], fp16, tag="t1", name=f"ti{t}_{ft}")
                    tf = tmpp.tile([128, NLOC], fp16, tag="t2", name=f"tf{t}_{ft}")
                    tt = tmpp.tile([128, NLOC], fp16, tag="t3", name=f"tt{t}_{ft}")
                    to = tmpp.tile([128, NLOC], fp16, tag="t4", name=f"to{t}_{ft}")
                    tc2 = tmpp.tile([128, NLOC], fp16, tag="t1", name=f"tc{t}_{ft}")
                    nc.scalar.activation(ti, gacc[:, 0 + ft, :], SIG,
                                         bias=sb_bias[:, 0 + ft:1 + ft])
                    nc.scalar.activation(tf, gacc[:, 2 + ft, :], SIG,
                                         bias=sb_bias[:, 2 + ft:3 + ft])
                    nc.scalar.activation(tt, gacc[:, 4 + ft, :], TANH,
                                         bias=sb_bias[:, 4 + ft:5 + ft])
                    nc.scalar.activation(to, gacc[:, 6 + ft, :], SIG,
                                         bias=sb_bias[:, 6 + ft:7 + ft])
                    if t == 0:
                        nc.vector.tensor_mul(c_new[:, ft, :], ti, tt)
                    else:
                        nc.vector.tensor_mul(ti, ti, tt)
                        nc.vector.tensor_mul(tf, tf, c_fm[:, ft, :])
                        nc.vector.tensor_add(c_new[:, ft, :], ti, tf)
                    nc.scalar.activation(tc2, c_new[:, ft, :], TANH)
                    nc.vector.tensor_mul(h_new[:, ft, :], to, tc2)
                    if not last:
                        # node-major own slice (feature half ft)
                        fs = slice(ft * 128, (ft + 1) * 128)
                        if dma_tr:
                            nc.sync.dma_start_transpose(hnm[:, :, fs],
                                                        h_new[:, ft, :])
                        else:
                            for kk in range(KLOC):
                                pt = psg.tile([128, 128], fp16, tag="tps",
                                              name=f"tp{t}_{ft}_{kk}", bufs=2)
                                nc.tensor.transpose(
                                    pt, h_new[:, ft, kk * 128:(kk + 1) * 128],
                                    ident)
                                nc.vector.tensor_copy(
                                    hnm[:, kk, ft * 128:(ft + 1) * 128], pt)
                        if split_ag:
                            emit_ag(ft)
                if not last and not split_ag:
                    emit_ag(0)
                h_fm, c_fm = h_new, c_new
                if not last:
                    hnm_prev = hnm

            nc.sync.dma_start(d_h.rearrange("f p n -> p f n"), h_fm)
            nc.sync.dma_start(d_c.rearrange("f p n -> p f n"), c_fm)

    nc.compile()
    return nc


def _host_prep(X, edge_weight, Ws, bs, thetas, conv_bs, edge_index, own_first=False):
    """Build per-core device inputs from the raw problem inputs."""
    src = edge_index[0].astype(np.int64)
    dst = edge_index[1].astype(np.int64)
    ew = edge_weight.astype(np.float32)
    deg = np.bincount(src, weights=ew, minlength=N)
    dis = np.where(deg > 0, 1.0 / np.sqrt(np.where(deg > 0, deg, 1.0)), 0.0)
    dis = dis.astype(np.float32)
    w_hat = ((2.0 / LAMBDA_MAX) * (-dis[src] * ew * dis[dst])).astype(np.float32)
    diag = np.float32(2.0 / LAMBDA_MAX - 1.0)
    L = np.zeros((N, N), np.float32)
    np.add.at(L, (dst, src), w_hat)
    if diag != 0.0:
        L[np.arange(N), np.arange(N)] += diag
    L2 = L @ L

    # Theta: rows [X; H; U; V] x cols [I|F|T|O]
    Th = np.zeros((4 * F, 4 * F), np.float32)
    bias_full = np.zeros(4 * F, np.float32)
    for g in range(4):
        cs = slice(g * F, (g + 1) * F)
        Th[0 * F:1 * F, cs] = Ws[g]
        Th[1 * F:2 * F, cs] = thetas[g, 0] - thetas[g, 2]
        Th[2 * F:3 * F, cs] = thetas[g, 1]
        Th[3 * F:4 * F, cs] = 2.0 * thetas[g, 2]
        bias_full[cs] = bs[g] + conv_bs[g]
    th_t = np.ascontiguousarray(Th.reshape(GM, 128, 4 * F).astype(np.float16))
    bias_t = np.ascontiguousarray(bias_full.reshape(GM, 128).astype(np.float32))

    in_maps = []
    for i in range(NCORES):
        rows = slice(i * NLOC, (i + 1) * NLOC)
        rhs = np.concatenate([L[rows].T, L2[rows].T], axis=1)  # [N, 1280]
        if own_first:
            own = rhs[rows].reshape(KLOC, 128, NOUT2)
            rest = rhs.copy()
            rest[rows] = 0.0
            ll2 = np.ascontiguousarray(np.concatenate(
                [own, rest.reshape(KT, 128, NOUT2)], axis=0).astype(np.float16))
        else:
            ll2 = np.ascontiguousarray(
                rhs.reshape(KT, 128, NOUT2).astype(np.float16))
        # reference uses Xs = X.reshape(N, T, F) (torch-.view semantics: raw
        # memory reinterpretation), node n's time series is row n of that view
        xi = np.ascontiguousarray(
            X.reshape(N, T, F)[rows].transpose(1, 2, 0)
            .reshape(T, FT, 128, NLOC).astype(np.float16))
        in_maps.append(dict(ll2=ll2, th=th_t, xall=xi, biasv=bias_t))
    return in_maps


def kernel(X, edge_weight, Ws, bs, thetas, conv_bs, edge_index):
    X = np.asarray(X, dtype=np.float32)
    edge_weight = np.asarray(edge_weight, dtype=np.float32)
    Ws = np.asarray(Ws, dtype=np.float32)
    bs = np.asarray(bs, dtype=np.float32)
    thetas = np.asarray(thetas, dtype=np.float32)
    conv_bs = np.asarray(conv_bs, dtype=np.float32)
    edge_index = np.asarray(edge_index)

    in_maps = _host_prep(X, edge_weight, Ws, bs, thetas, conv_bs, edge_index)
    if "nc" not in _CACHE:
        _CACHE["nc"] = _build_nc()
    nc = _CACHE["nc"]
    res = run_bass_kernel_spmd(nc, in_maps, core_ids=list(range(NCORES)))

    H = np.empty((N, F), np.float32)
    C = np.empty((N, F), np.float32)
    for i in range(NCORES):
        rows = slice(i * NLOC, (i + 1) * NLOC)
        H[rows] = res.results[i]["hout"].reshape(F, NLOC).T
        C[rows] = res.results[i]["cout"].reshape(F, NLOC).T
    return H, C

